# revision 2
# baseline (speedup 1.0000x reference)
"""Trainium2 Bass kernel v2 for complex-valued spatial-reduction attention.

x: [B=4, N=2304, C=512] complex64, H=W=48, 8 heads, hd=64, sr=2 -> Nk=576.
Sharding: 8 cores = 4 batches x 2 head-groups (4 heads each).

v2 restructure vs baseline:
- conv emitted directly in [c_out, nk] layout (no PE transposes/staging).
- complex LayerNorm fully FOLDED: per-token m (mean) and alpha=1/sqrt(var+eps)
  commute through the kv projection: k = alpha*(Wk^T y - m*u) + bk with
  u = colsum(Wk). The -m*u / (1/alpha)*bk terms are rank-1 matmuls into the
  kv PSUM; the complex alpha is applied on DVE (broadcast tiles for k,
  per-partition scalars for v). No elementwise LN over [C, Nk] at all.
- scores: q per head stored [qr;qi] on partitions, k as kA=[kr;-ki],
  kB=[ki;kr]: each score component is ONE K=128 matmul (half the rows).
- softmax chain in f32: custom DVE op SQ2 (sre^2+sim^2 in one pass from two
  PSUM banks), ACT sqrt in-place, ACT exp in-place (bitcast f32r), batched
  per q-chunk so each needs only two ACT table loads.
- denominator: partition-packed PSUM groups; reciprocal on [1,nq] rows;
  broadcast via gpsimd partition_broadcast (no PE broadcast matmuls).
- f32 output partials DMA'd straight from PSUM; host sums core pairs.
"""

import os
import contextlib

import numpy as np

import concourse.bass as bass
import concourse.mybir as mybir
import concourse.tile as tile
from concourse import bacc
from concourse.masks import make_identity

F32 = mybir.dt.float32
F32R = mybir.dt.float32r
AF = mybir.ActivationFunctionType
ALU = mybir.AluOpType

B, N, C, HEADS, HD, SR = 4, 2304, 512, 8, 64, 2
NK = 576
HR = 24
EPS = 1e-5
SCALE = HD ** -0.5  # folded into Wk host-side

K_CHUNKS = [(0, 128), (128, 128), (256, 128), (384, 128), (512, 64)]
Q_CHUNKS = [(0, 512), (512, 512), (1024, 512), (1536, 512), (2048, 256)]
NKH = [(0, 288), (288, 288)]

KDBG = int(os.environ.get("KDBG", "0"))
KSQ2 = int(os.environ.get("KSQ2", "1"))   # 1: custom DVE sq2 op
KPBC = int(os.environ.get("KPBC", "1"))   # 1: gpsimd partition_broadcast


# ---------------------------------------------------------------------------
# custom DVE op: out = in0^2 + in1^2 (one pass; both inputs may be PSUM)
# ---------------------------------------------------------------------------
_SQ2 = None


def _get_sq2():
    global _SQ2
    if _SQ2 is not None:
        return _SQ2
    import re
    from concourse import dve_ops
    from concourse.dve_spec import Spec, Src0, Src1, sq

    for o in dve_ops.OPS:
        if o.name == "SQ2_ANT":
            _SQ2 = o
            return o
    op = dve_ops.DveOp(
        "SQ2_ANT",
        Spec(body=sq(Src0) + sq(Src1),
             reference=lambda in0, in1, s0, s1, imm2: (
                 np.asarray(in0, np.float32) ** 2
                 + np.asarray(in1, np.float32) ** 2)),
        subdim=False, uops_sha={})
    dve_ops.OPS.append(op)
    dve_ops.CUSTOM_DVE_SPECS[op.name] = op.spec
    dve_ops._SUB_OPCODE_FOR_NAME[op.name] = (
        max(dve_ops._SUB_OPCODE_FOR_NAME.values()) + 1)
    for ver in ("v3", "v4"):
        try:
            op.compile(ver)
        except ValueError as e:
            m = re.search(r"(v\d): ([0-9a-f]{16})", str(e))
            op.uops_sha[m.group(1)] = m.group(2)
            op.compile(ver)
    _SQ2 = op
    return op


def build_nc():
    nc = bacc.Bacc("TRN2", target_bir_lowering=False, debug=False,
                   num_devices=8)

    xP_d = nc.dram_tensor("xP", [2, 16, 128, NK], F32R, kind="ExternalInput")
    wc_d = nc.dram_tensor("wc", [3, 16, 4, 128, 128], F32R, kind="ExternalInput")
    srb_d = nc.dram_tensor("srb", [2, 4, 128, 1], F32, kind="ExternalInput")
    onesc_d = nc.dram_tensor("onesc", [128, 1], F32R, kind="ExternalInput")
    wq_d = nc.dram_tensor("wq", [4, 2, 4, 128, 128], F32R, kind="ExternalInput")
    wk_d = nc.dram_tensor("wk", [4, 2, 4, 128, 128], F32R, kind="ExternalInput")
    wv_d = nc.dram_tensor("wv", [3, 4, 128, 256], F32R, kind="ExternalInput")
    wp_d = nc.dram_tensor("wp", [3, 2, 128, 512], F32R, kind="ExternalInput")
    uk_d = nc.dram_tensor("uk", [4, 4, 1, 128], F32R, kind="ExternalInput")
    uv_d = nc.dram_tensor("uv", [6, 1, 256], F32R, kind="ExternalInput")
    outT_d = nc.dram_tensor("outT", [2, 4, 5, 128, 512], F32,
                            kind="ExternalOutput")
    dbg = {}
    if KDBG:
        dbg["xconv"] = nc.dram_tensor("dbg_xconv", [2, C, NK], F32,
                                      kind="ExternalOutput")
        dbg["alpha"] = nc.dram_tensor("dbg_alpha", [6, NK], F32,
                                      kind="ExternalOutput")
        dbg["kAB"] = nc.dram_tensor("dbg_kAB", [2, 4, 128, NK], F32,
                                    kind="ExternalOutput")
        dbg["vpk"] = nc.dram_tensor("dbg_vpk", [128, 5 * 4 * 128], F32,
                                    kind="ExternalOutput")
        dbg["q"] = nc.dram_tensor("dbg_q", [4, 128, N], F32,
                                  kind="ExternalOutput")

    with tile.TileContext(nc) as tc:
        _body(nc, tc, xP_d, wc_d, srb_d, onesc_d, wq_d, wk_d, wv_d, wp_d,
              uk_d, uv_d, outT_d, dbg)

    nc.compile()
    # the stock walrus birverifier asserts on custom ISA rows (SQ2_ANT,
    # PartitionBroadcast); codegen emits the pre-encoded bytes fine, so
    # skip verification for those instructions.
    for f in nc.m.functions:
        for blk in f.blocks:
            for inst in blk.instructions:
                if isinstance(inst, mybir.InstISA) and inst.verify:
                    inst.verify = False
    return nc


def _sqrt_newton(nc, pool, out, x, sz, sc, tagp):
    """out = sqrt(sc*max(x,0)), ACT LUT seed + one Newton step."""
    nc.vector.tensor_scalar_max(x[:, :sz], x[:, :sz], 0.0)
    y0 = pool.tile([128, 8], F32, tag=f"{tagp}_y0", name=f"{tagp}_y0")
    nc.scalar.activation(y0[:, :sz], x[:, :sz], AF.Sqrt, scale=sc)
    yr = pool.tile([128, 8], F32, tag=f"{tagp}_yr", name=f"{tagp}_yr")
    nc.vector.tensor_scalar_max(y0[:, :sz], y0[:, :sz], 1e-12)
    nc.vector.reciprocal_approx_fast(yr[:, :sz], y0[:, :sz])
    nc.vector.tensor_mul(yr[:, :sz], yr[:, :sz], x[:, :sz])
    if sc != 1.0:
        nc.vector.tensor_scalar_mul(yr[:, :sz], yr[:, :sz], sc)
    nc.vector.tensor_add(out[:, :sz], y0[:, :sz], yr[:, :sz])
    nc.vector.tensor_scalar_mul(out[:, :sz], out[:, :sz], 0.5)


def _body(nc, tc, xP_d, wc_d, srb_d, onesc_d, wq_d, wk_d, wv_d, wp_d,
          uk_d, uv_d, outT_d, dbg):
    SQ2 = _get_sq2() if KSQ2 else None
    ctx = contextlib.ExitStack()
    consts = ctx.enter_context(tc.tile_pool(name="consts", bufs=1))
    big = ctx.enter_context(tc.tile_pool(name="big", bufs=1))

    ident = consts.tile([128, 128], F32, tag="ident", name="ident")
    make_identity(nc, ident)
    onesc = consts.tile([128, 1], F32R, tag="onesc", name="onesc")
    nc.sync.dma_start(onesc[:], onesc_d[:, :])
    nbias = consts.tile([128, 1], F32, tag="nbias", name="nbias")
    nc.vector.memset(nbias, -50.0)
    onesr_c = consts.tile([1, 128], F32R, tag="onesrc", name="onesr_c")
    nc.vector.memset(onesr_c.bitcast(F32), 1.0)
    srb_sb = [[consts.tile([128, 1], F32, tag=f"srb{p}{t}",
                           name=f"srb{p}{t}") for t in range(4)]
              for p in range(2)]
    for p in range(2):
        for t in range(4):
            nc.sync.dma_start(srb_sb[p][t][:], srb_d[p, t])

    # whole-kernel SBUF residents (kept minimal: SBUF is tight in phase A)
    q_sb = big.tile([128, 4, N], F32R, tag="q_sb", name="q_sb")
    aw = big.tile([128, 5, 16], F32, tag="aw", name="aw")

    # =====================================================================
    # Phase A: conv in [c_out, nk] layout + q-projection interleaved
    # =====================================================================
    if True:
        bigA = ctx.enter_context(tc.tile_pool(name="bigA", bufs=1))
        xconv = bigA.tile([128, 2, 4, NK], F32R, tag="xconv", name="xconv")
        stats_part = bigA.tile([1, 5, NK], F32, tag="spart", name="spart")

        with tc.tile_pool(name="xpA", bufs=1) as xpA, \
             tc.tile_pool(name="cwork", bufs=2) as cwork, \
             tc.tile_pool(name="sqp", bufs=1) as sqp, \
             tc.tile_pool(name="cvps", bufs=4, space="PSUM") as cvps, \
             tc.tile_pool(name="qpps", bufs=2, space="PSUM") as qpps, \
             tc.tile_pool(name="stps", bufs=2, space="PSUM") as stps:

            xP = xpA.tile([128, 2, 16, NK], F32R, tag="xP", name="xP")
            wq = xpA.tile([128, 4, 2, 4, 128], F32R, tag="wq", name="wq")
            for h in range(4):
                for mv in range(2):
                    for cj in range(4):
                        nc.gpsimd.dma_start(wq[:, h, mv, cj, :],
                                            wq_d[h, mv, cj])

            def emit_q(p4):
                for h in range(4):
                    prs = []
                    for ni in range(2):
                        prs.append(qpps.tile([128, 288], F32, tag="qp",
                                             name=f"q{p4}_{h}_{ni}"))
                    for mv in range(2):
                        for cj in range(4):
                            st = mv == 0 and cj == 0
                            sp = mv == 1 and cj == 3
                            for ni, (n0, nn) in enumerate(NKH):
                                nc.tensor.matmul(
                                    prs[ni][:, :nn], wq[:, h, mv, cj, :],
                                    xP[:, mv, 4 * p4 + cj, n0:n0 + nn],
                                    start=st, stop=sp)
                    with nc.allow_low_precision(reason="f32r rounding"):
                        for ni, (n0, nn) in enumerate(NKH):
                            q0 = p4 * NK + n0
                            nc.vector.tensor_copy(
                                q_sb[:, h, q0:q0 + nn], prs[ni][:, :nn])

            for t in range(4):
                cps = []
                for pl in range(2):
                    for ni in range(2):
                        cps.append(cvps.tile([128, 288], F32, tag="cv",
                                             name=f"cv{t}_{pl}{ni}"))
                cs = slice(128 * t, 128 * (t + 1))
                for kk in range(16):
                    if t == 0:
                        nc.sync.dma_start(xP[:, 0, kk, :], xP_d[0, kk])
                        nc.gpsimd.dma_start(xP[:, 1, kk, :], xP_d[1, kk])
                    wcr = cwork.tile([128, 128], F32R, tag="wc_r", name="wcr")
                    wci = cwork.tile([128, 128], F32R, tag="wc_i", name="wci")
                    wcm = cwork.tile([128, 128], F32R, tag="wc_m", name="wcm")
                    nc.sync.dma_start(wcr[:], wc_d[0, kk, t])
                    nc.gpsimd.dma_start(wci[:], wc_d[1, kk, t])
                    nc.sync.dma_start(wcm[:], wc_d[2, kk, t])
                    st = kk == 0
                    sp = kk == 15
                    for ni, (n0, nn) in enumerate(NKH):
                        xr = xP[:, 0, kk, n0:n0 + nn]
                        xi = xP[:, 1, kk, n0:n0 + nn]
                        cr = cps[ni]
                        ci = cps[2 + ni]
                        nc.tensor.matmul(cr[:, :nn], wcr[:], xr, start=st,
                                         stop=False)
                        nc.tensor.matmul(ci[:, :nn], wcr[:], xi, start=st,
                                         stop=False)
                        nc.tensor.matmul(ci[:, :nn], wci[:], xr, start=False,
                                         stop=sp)
                        nc.tensor.matmul(cr[:, :nn], wcm[:], xi, start=False,
                                         stop=sp)
                    if t == 0 and kk == 7:
                        emit_q(0)
                # epilogue: copy + sr_b bias into SBUF, squares, stat partials
                with nc.allow_low_precision(reason="f32r rounding modeled"):
                    for pl in range(2):
                        for ni, (n0, nn) in enumerate(NKH):
                            nc.vector.tensor_scalar_add(
                                xconv[:, pl, t, n0:n0 + nn],
                                cps[2 * pl + ni][:, :nn], srb_sb[pl][t][:])
                    sq = sqp.tile([128, 3, NK], F32R, tag="sq",
                                  name=f"sq{t}")
                    nc.scalar.activation(sq[:, 0, :], xconv[:, 0, t, :],
                                         AF.Square)
                    nc.scalar.activation(sq[:, 1, :], xconv[:, 1, t, :],
                                         AF.Square)
                    nc.vector.tensor_mul(sq[:, 2, :], xconv[:, 0, t, :],
                                         xconv[:, 1, t, :])
                movs = [xconv[:, 0, t, :],
                        xconv[:, 1, t, :],
                        sq[:, 0, :],
                        sq[:, 1, :],
                        sq[:, 2, :]]
                for qy in range(5):
                    for ni, (n0, nn) in enumerate(NKH):
                        sps = stps.tile([128, 288], F32, tag="st",
                                        name=f"st{t}_{qy}_{ni}")
                        nc.tensor.matmul(sps[0:1, :nn], onesc[:],
                                         movs[qy][:, n0:n0 + nn],
                                         start=True, stop=True)
                        if t == 0:
                            nc.vector.tensor_copy(
                                stats_part[:, qy, n0:n0 + nn],
                                sps[0:1, :nn])
                        else:
                            nc.vector.tensor_add(
                                stats_part[:, qy, n0:n0 + nn],
                                stats_part[:, qy, n0:n0 + nn],
                                sps[0:1, :nn])
                if t < 2:
                    emit_q(t + 1)
            emit_q(3)

        # ---- kv-weight DMAs (overlap the alpha chain) ----
        bigBC = ctx.enter_context(tc.tile_pool(name="bigBC", bufs=1))
        kAf = bigBC.tile([128, 4, NK], F32R, tag="kAf", name="kAf")
        kBf = bigBC.tile([128, 4, NK], F32R, tag="kBf", name="kBf")
        vpk = bigBC.tile([128, 5, 4, 128], F32R, tag="vpk", name="vpk")
        wp = bigBC.tile([128, 3, 2, 512], F32R, tag="wp", name="wp")
        uk = bigBC.tile([1, 4, 4, 128], F32R, tag="uk", name="uk")
        uv = bigBC.tile([1, 6, 256], F32R, tag="uv", name="uv")
        with tc.tile_pool(name="poolB", bufs=1) as poolB, \
             tc.tile_pool(name="tpps", bufs=2, space="PSUM") as tpps:
            wk = poolB.tile([128, 4, 2, 4, 128], F32R, tag="wk", name="wk")
            wv = poolB.tile([128, 3, 4, 256], F32R, tag="wv", name="wv")
            arb = poolB.tile([128, 2, NK], F32R, tag="arb", name="arb")
            arow_all = poolB.tile([1, 6, NK], F32R, tag="arows",
                                  name="arow_all")
            for h in range(4):
                for mv in range(2):
                    for cj in range(4):
                        eng = nc.sync if cj % 2 == 0 else nc.gpsimd
                        eng.dma_start(wk[:, h, mv, cj, :], wk_d[h, mv, cj])
            for pl in range(3):
                for cj in range(4):
                    nc.gpsimd.dma_start(wv[:, pl, cj, :], wv_d[pl, cj])
                for hp in range(2):
                    nc.sync.dma_start(wp[:, pl, hp, :], wp_d[pl, hp])
            for h in range(4):
                for j in range(4):
                    nc.sync.dma_start(uk[:, h, j, :], uk_d[h, j])
            for j in range(6):
                nc.gpsimd.dma_start(uv[:, j, :], uv_d[j])

            stats_sb = stats_part

            # ---- alpha/m math on token-partitions ----
            for c5 in range(5):
                k0, szk = K_CHUNKS[c5]
                tp = tpps.tile([128, 128], F32, tag="tp", name=f"tp{c5}")
                for qy in range(5):
                    nc.tensor.transpose(tp[:szk, qy:qy + 1],
                                        stats_sb[:, qy, k0:k0 + szk],
                                        ident[:1, :1])
                nc.vector.tensor_copy(aw[:szk, c5, 0:5], tp[:szk, 0:5])

            inv_c = 1.0 / C
            A = lambda q: aw[:, :, q]
            awp = lambda q: aw[:, :, q:q + 1].rearrange("p a b -> p (a b)")
            # slots: 0..4 sums; 5=mr 6=mi 7=sr 8=si 9=ar 10=ai
            # 11,12 scratch; 13=vre 14=vim 15=|v|
            nc.vector.tensor_scalar_mul(A(5), A(0), inv_c)
            nc.vector.tensor_scalar_mul(A(6), A(1), inv_c)
            nc.vector.tensor_sub(A(13), A(2), A(3))
            nc.vector.tensor_scalar_mul(A(13), A(13), inv_c)
            nc.vector.tensor_mul(A(11), A(5), A(5))
            nc.vector.tensor_mul(A(12), A(6), A(6))
            nc.vector.tensor_sub(A(11), A(11), A(12))
            nc.vector.tensor_sub(A(13), A(13), A(11))
            nc.vector.tensor_scalar_add(A(13), A(13), EPS)   # vre
            nc.vector.tensor_mul(A(11), A(5), A(6))
            nc.vector.tensor_scalar_mul(A(11), A(11), 2.0)
            nc.vector.tensor_scalar_mul(A(14), A(4), 2.0 * inv_c)
            nc.vector.tensor_sub(A(14), A(14), A(11))        # vim
            nc.vector.tensor_mul(A(11), A(13), A(13))
            nc.vector.tensor_mul(A(12), A(14), A(14))
            nc.vector.tensor_add(A(11), A(11), A(12))        # |v|^2
            _sqrt_newton(nc, consts, awp(15), awp(11), 5, 1.0, "nv")
            nc.vector.tensor_add(A(11), A(15), A(13))
            _sqrt_newton(nc, consts, awp(7), awp(11), 5, 0.5, "nr")
            nc.vector.tensor_sub(A(11), A(15), A(13))
            _sqrt_newton(nc, consts, awp(8), awp(11), 5, 0.5, "ni")
            sgn = consts.tile([128, 8], F32, tag="sgn", name="sgn")
            nc.scalar.activation(sgn[:, 0:5], A(14), AF.Sign)
            nc.vector.tensor_mul(A(8), A(8), sgn[:, 0:5])             # si
            rin = consts.tile([128, 8], F32, tag="rin", name="rin")
            nc.vector.reciprocal_approx_fast(rin[:, 0:5], A(15))
            nc.vector.tensor_mul(A(9), A(7), rin[:, 0:5])             # ar
            nc.vector.tensor_mul(A(10), A(8), rin[:, 0:5])
            nc.vector.tensor_scalar_mul(A(10), A(10), -1.0)           # ai

            # transpose back -> arow_all rows (mr mi sr si ar ai)
            for c5 in range(5):
                k0, szk = K_CHUNKS[c5]
                for j in range(6):
                    tpb = tpps.tile([128, 128], F32, tag="tp",
                                    name=f"tpb{c5}_{j}")
                    nc.tensor.transpose(tpb[0:1, :szk],
                                        aw[:szk, c5, 5 + j:6 + j],
                                        ident[:szk, :szk])
                    with nc.allow_low_precision(reason="f32r rounding"):
                        nc.vector.tensor_copy(arow_all[:, j, k0:k0 + szk],
                                              tpb[0:1, :szk])
            if KPBC:
                nc.gpsimd.partition_broadcast(arb[:, 0, :],
                                              arow_all[:, 4, :])
                nc.gpsimd.partition_broadcast(arb[:, 1, :],
                                              arow_all[:, 5, :])
            else:
                onesr = consts.tile([1, 128], F32R, tag="onesr",
                                    name="onesr")
                nc.vector.memset(onesr.bitcast(F32), 1.0)
                for j, (a0, aa) in enumerate(((4, 0), (5, 1))):
                    for n0, nn in ((0, 512), (512, 64)):
                        bp = tpps.tile([128, 512], F32, tag="bp",
                                       name=f"bp{j}")
                        nc.tensor.matmul(bp[:, :nn], onesr[:],
                                         arow_all[:, a0, n0:n0 + nn],
                                         start=True, stop=True)
                        nc.vector.tensor_copy(arb[:, aa, n0:n0 + nn],
                                              bp[:, :nn])

            # =============================================================
            # Phase B: k (kA/kB + rank-1 folds + alpha), v (+alpha)
            # =============================================================
            with tc.tile_pool(name="kvps", bufs=2, space="PSUM") as kvps, \
                 tc.tile_pool(name="vps", bufs=4 if KPBC else 2, space="PSUM") as vps, \
                 tc.tile_pool(name="kw", bufs=1) as kw:
                ar_b = arb.bitcast(F32)[:, 0, :]
                ai_b = arb.bitcast(F32)[:, 1, :]
                arr = lambda j: arow_all[:, j, :]
                for h in range(4):
                    kps = []
                    for ni in range(2):
                        kps.append(kvps.tile([128, 288], F32, tag="kp",
                                             name=f"k{h}_{ni}"))
                    for mv in range(2):
                        for cj in range(4):
                            for ni, (n0, nn) in enumerate(NKH):
                                nc.tensor.matmul(
                                    kps[ni][:, :nn], wk[:, h, mv, cj, :],
                                    xconv[:, mv, cj,
                                                        n0:n0 + nn],
                                    start=(mv == 0 and cj == 0), stop=False)
                    for j in range(4):
                        for ni, (n0, nn) in enumerate(NKH):
                            nc.tensor.matmul(
                                kps[ni][:, :nn], uk[:, h, j, :],
                                arr(j)[:, n0:n0 + nn],
                                start=False, stop=j == 3)
                    for ni, (n0, nn) in enumerate(NKH):
                        kat = kw.tile([128, 288], F32, tag="kat",
                                      name=f"kat{h}_{ni}")
                        kbt = kw.tile([128, 288], F32, tag="kbt",
                                      name=f"kbt{h}_{ni}")
                        t1 = kw.tile([128, 288], F32, tag="kt1",
                                     name=f"kt1_{h}_{ni}")
                        t2 = kw.tile([128, 288], F32, tag="kt2",
                                     name=f"kt2_{h}_{ni}")
                        ns = slice(n0, n0 + nn)
                        ab_r = arb.bitcast(F32)[:, 0, ns]
                        ab_i = arb.bitcast(F32)[:, 1, ns]
                        nc.vector.tensor_copy(kat[:, :nn], kps[ni][:, :nn])
                        nc.vector.tensor_scalar_mul(kbt[0:64, :nn],
                                                    kat[64:128, :nn], -1.0)
                        nc.vector.tensor_copy(kbt[64:128, :nn],
                                              kat[0:64, :nn])
                        nc.vector.tensor_mul(t1[:, :nn], ab_r, kat[:, :nn])
                        nc.vector.tensor_mul(t2[:, :nn], ab_i, kbt[:, :nn])
                        with nc.allow_low_precision(reason="f32r rounding"):
                            nc.vector.tensor_sub(kAf[:, h, ns], t1[:, :nn],
                                                 t2[:, :nn])
                        nc.vector.tensor_mul(t1[:, :nn], ab_r, kbt[:, :nn])
                        nc.vector.tensor_mul(t2[:, :nn], ab_i, kat[:, :nn])
                        with nc.allow_low_precision(reason="f32r rounding"):
                            nc.vector.tensor_add(kBf[:, h, ns], t1[:, :nn],
                                                 t2[:, :nn])

                # ---- v ----
                stt = nc.vector.scalar_tensor_tensor
                for kc, (k0, szk) in enumerate(K_CHUNKS):
                    vr = vps.tile([128, 256], F32, tag="vp", name=f"vr{kc}")
                    vi = vps.tile([128, 256], F32, tag="vp", name=f"vi{kc}")
                    for cj in range(4):
                        xr = xconv[:, 0, cj, k0:k0 + szk]
                        xi = xconv[:, 1, cj, k0:k0 + szk]
                        st = cj == 0
                        nc.tensor.matmul(vr[:szk, :], xr, wv[:, 0, cj, :],
                                         start=st, stop=False)
                        nc.tensor.matmul(vi[:szk, :], xr, wv[:, 1, cj, :],
                                         start=st, stop=False)
                        nc.tensor.matmul(vr[:szk, :], xi, wv[:, 2, cj, :],
                                         start=False, stop=False)
                        nc.tensor.matmul(vi[:szk, :], xi, wv[:, 0, cj, :],
                                         start=False, stop=False)
                    vr_terms = [(0, 0), (1, 1), (2, 3), (3, 4)]
                    vi_terms = [(0, 2), (1, 0), (2, 5), (3, 3)]
                    for i, (arow, uvrow) in enumerate(vr_terms):
                        nc.tensor.matmul(vr[:szk, :],
                                         arr(arow)[:, k0:k0 + szk],
                                         uv[:, uvrow, :], start=False,
                                         stop=i == 3)
                    for i, (arow, uvrow) in enumerate(vi_terms):
                        nc.tensor.matmul(vi[:szk, :],
                                         arr(arow)[:, k0:k0 + szk],
                                         uv[:, uvrow, :], start=False,
                                         stop=i == 3)
                    # alpha apply (per-partition scalars from aw chunk kc)
                    tmr = kw.tile([128, 256], F32, tag="tmr", name=f"tmr{kc}")
                    tmi = kw.tile([128, 256], F32, tag="tmi", name=f"tmi{kc}")
                    ar_s = awp(9)[:, kc:kc + 1]
                    ai_s = awp(10)[:, kc:kc + 1]
                    nc.vector.tensor_scalar_mul(tmr[:szk], vi[:szk, :],
                                                ai_s[:szk])
                    nc.vector.tensor_scalar_mul(tmi[:szk], vr[:szk, :],
                                                ai_s[:szk])
                    vout_r = vpk[:szk, kc, :, 0:64]
                    vout_i = vpk[:szk, kc, :, 64:128]
                    vrr = vr[:szk, :].rearrange("p (h d) -> p h d", h=4)
                    vir = vi[:szk, :].rearrange("p (h d) -> p h d", h=4)
                    tmrr = tmr[:szk].rearrange("p (h d) -> p h d", h=4)
                    tmir = tmi[:szk].rearrange("p (h d) -> p h d", h=4)
                    with nc.allow_low_precision(reason="f32r rounding"):
                        stt(vout_r, vrr, ar_s[:szk], tmrr, ALU.mult,
                            ALU.subtract)
                        stt(vout_i, vir, ar_s[:szk], tmir, ALU.mult, ALU.add)

                if KDBG:
                    for pl in range(2):
                        for t in range(4):
                            nc.sync.dma_start(
                                dbg["xconv"][pl, 128 * t:128 * (t + 1), :],
                                xconv.bitcast(F32)[:, pl, t, :])
                    for j in range(6):
                        nc.sync.dma_start(dbg["alpha"][j:j + 1, :],
                                          arow_all.bitcast(F32)[:, j, :])
                    for h in range(4):
                        nc.sync.dma_start(dbg["kAB"][0, h],
                                          kAf.bitcast(F32)[:, h, :])
                        nc.sync.dma_start(dbg["kAB"][1, h],
                                          kBf.bitcast(F32)[:, h, :])
                        nc.sync.dma_start(dbg["q"][h],
                                          q_sb.bitcast(F32)[:, h, :])
                    nc.sync.dma_start(
                        dbg["vpk"][:, :],
                        vpk.bitcast(F32).rearrange("p a b c -> p (a b c)"))

    # =====================================================================
    # Phase C: attention + projection, software-pipelined over q-chunks
    # =====================================================================
    with tc.tile_pool(name="sm", bufs=1) as sm, \
         tc.tile_pool(name="scps", bufs=2, space="PSUM") as scps, \
         tc.tile_pool(name="ovps", bufs=2, space="PSUM") as ovps, \
         tc.tile_pool(name="dnps", bufs=2, space="PSUM") as dnps, \
         tc.tile_pool(name="pjps", bufs=2, space="PSUM") as pjps:

        def emit_front_h(qi, q0, nq, h, stiles):
            sh = sm.tile([128, 5, 512], F32R, tag="s", bufs=5, name=f"s{h}")
            stiles[h] = sh
            for kc, (k0, szk) in enumerate(K_CHUNKS):
                sre = scps.tile([128, 512], F32, tag="sc",
                                name=f"sre{h}_{kc}")
                sim = scps.tile([128, 512], F32, tag="sc",
                                name=f"sim{h}_{kc}")
                nc.tensor.matmul(sre[:szk, :nq], kAf[:, h, k0:k0 + szk],
                                 q_sb[:, h, q0:q0 + nq], start=True,
                                 stop=True)
                nc.tensor.matmul(sim[:szk, :nq], kBf[:, h, k0:k0 + szk],
                                 q_sb[:, h, q0:q0 + nq], start=True,
                                 stop=True)
                t2 = sm.tile([128, 512], F32, tag="t2", bufs=2, name="t2")
                nc.vector.tensor_copy(t2[:szk, :nq], sim[:szk, :nq])
                with nc.allow_low_precision(reason="pre-sqrt"):
                    nc.vector._custom_dve(SQ2, out=sh[:szk, kc, :nq],
                                          in0=sre[:szk, :nq],
                                          in1=t2[:szk, :nq])

        def emit_act(qi, nq, stiles):
            # |a| = exp(0.5*ln(s)); ebuf = exp(|a| - 50): ln+exp share one
            # ACT table set (natural_log_exp_and_others).
            for h in range(4):
                sh = stiles[h]
                nc.scalar.activation(sh[:, :, :nq], sh[:, :, :nq], AF.Ln)
            for h in range(4):
                sh = stiles[h]
                nc.scalar.activation(sh[:, :, :nq], sh[:, :, :nq], AF.Exp,
                                     scale=0.5)
            for h in range(4):
                sh = stiles[h]
                nc.scalar.activation(sh[:, :, :nq], sh[:, :, :nq], AF.Exp,
                                     bias=nbias[:])

        ot_store = {}

        def emit_back_h(qi, q0, nq, stiles, h):
            ov = ovps.tile([128, 512], F32, tag="ov", name=f"ov{h}")
            dn_ps = dnps.tile([128, 512], F32, tag="dn", name=f"dn{h}")
            sh = stiles[h]
            for kc, (k0, szk) in enumerate(K_CHUNKS):
                eb = sh[:, kc, :]
                nc.tensor.matmul(ov[:, :nq], vpk[:szk, kc, h, :],
                                 eb[:szk, :nq], start=kc == 0,
                                 stop=kc == 4)
                nc.tensor.matmul(dn_ps[0:1, :nq], onesc[:szk],
                                 eb[:szk, :nq], start=kc == 0,
                                 stop=kc == 4)
            dnr = sm.tile([1, 512], F32, tag="dnr", bufs=2, name=f"dnr{h}")
            nc.scalar.copy(dnr[:, :nq], dn_ps[0:1, :nq])
            dri = sm.tile([1, 512], F32, tag="dri", bufs=2, name=f"dri{h}")
            nc.vector.reciprocal_approx_fast(dri[:, :nq], dnr[:, :nq])
            rb = sm.tile([128, 512], F32, tag="rb", bufs=2, name=f"rb{h}")
            if KPBC:
                nc.gpsimd.partition_broadcast(rb[:, :nq], dri[:, :nq])
            else:
                rbp = ovps.tile([128, 512], F32, tag="ov", name=f"rbp{h}")
                nc.tensor.matmul(rbp[:, :nq], onesr_c[:],
                                 dri.bitcast(F32R)[:, :nq],
                                 start=True, stop=True)
                nc.vector.tensor_copy(rb[:, :nq], rbp[:, :nq])
            hp, hi = h // 2, h % 2
            if hi == 0:
                ot_store[(qi, hp)] = (
                    sm.tile([128, 512], F32R, tag="otr", bufs=3,
                            name=f"otr{hp}"),
                    sm.tile([128, 512], F32R, tag="oti", bufs=3,
                            name=f"oti{hp}"))
            otr, oti = ot_store[(qi, hp)]
            rs = slice(64 * hi, 64 * (hi + 1))
            with nc.allow_low_precision(reason="f32r rounding"):
                nc.vector.tensor_mul(otr[rs, :nq], ov[0:64, :nq],
                                     rb[0:64, :nq])
                nc.vector.tensor_mul(oti[rs, :nq], ov[64:128, :nq],
                                     rb[64:128, :nq])

        def emit_proj(qi, q0, nq):
            for cc in range(4):
                cs = slice(128 * cc, 128 * (cc + 1))
                pr = pjps.tile([128, 512], F32, tag="pj", name=f"pr{cc}")
                pi = pjps.tile([128, 512], F32, tag="pj", name=f"pi{cc}")
                for hp in range(2):
                    otr, oti = ot_store[(qi, hp)]
                    st = hp == 0
                    sp = hp == 1
                    nc.tensor.matmul(pr[:, :nq], wp[:, 0, hp, cs],
                                     otr[:, :nq], start=st, stop=False)
                    nc.tensor.matmul(pi[:, :nq], wp[:, 0, hp, cs],
                                     oti[:, :nq], start=st, stop=False)
                    nc.tensor.matmul(pr[:, :nq], wp[:, 2, hp, cs],
                                     oti[:, :nq], start=False, stop=sp)
                    nc.tensor.matmul(pi[:, :nq], wp[:, 1, hp, cs],
                                     otr[:, :nq], start=False, stop=sp)
                o1 = sm.tile([128, 512], F32, tag="o1", bufs=2,
                             name="o1")
                o2 = sm.tile([128, 512], F32, tag="o2", bufs=2,
                             name="o2")
                nc.scalar.copy(o1[:, :nq], pr[:, :nq])
                nc.scalar.copy(o2[:, :nq], pi[:, :nq])
                eng = nc.sync if cc % 2 == 0 else nc.gpsimd
                eng2 = nc.gpsimd if cc % 2 == 0 else nc.sync
                eng.dma_start(outT_d[0, cc, qi, :, :nq], o1[:, :nq])
                eng2.dma_start(outT_d[1, cc, qi, :, :nq], o2[:, :nq])
            del ot_store[(qi, 0)]
            del ot_store[(qi, 1)]

        prev = None
        for qi, (q0, nq) in enumerate(Q_CHUNKS):
            stiles = {}
            for h in range(4):
                emit_front_h(qi, q0, nq, h, stiles)
                if prev is not None:
                    pqi, pq0, pnq, pst = prev
                    emit_back_h(pqi, pq0, pnq, pst, h)
            emit_act(qi, nq, stiles)
            if prev is not None:
                emit_proj(pqi, pq0, pnq)
            prev = (qi, q0, nq, stiles)
        pqi, pq0, pnq, pst = prev
        for h in range(4):
            emit_back_h(pqi, pq0, pnq, pst, h)
        emit_proj(pqi, pq0, pnq)

    ctx.close()


# =========================================================================
# Host side
# =========================================================================

def _f32(x):
    return np.ascontiguousarray(x, dtype=np.float32)


def _perm():
    perm = np.empty(4 * NK, dtype=np.int64)
    for p4 in range(4):
        p, q = p4 // 2, p4 % 2
        for nk in range(NK):
            hi, wi = nk // HR, nk % HR
            perm[p4 * NK + nk] = (SR * hi + p) * (SR * HR) + SR * wi + q
    return perm


_PERM = _perm()


def host_prep(x_re, x_im, Wq, Wkv, Wproj, bproj, sr_w, sr_b, gain, bias):
    x_re = np.asarray(x_re)
    x_im = np.asarray(x_im)
    Wq = np.asarray(Wq)
    Wkv = np.asarray(Wkv)
    Wproj = np.asarray(Wproj)
    sr_w = np.asarray(sr_w)
    sr_b = np.asarray(sr_b)
    gain = np.asarray(gain)
    bias = np.asarray(bias)

    Wkv_eff = gain[:, None] * Wkv
    bkv_full = bias @ Wkv
    Wc = sr_w.transpose(2, 3, 1, 0).reshape(4 * C, C)
    wc_pack = np.ascontiguousarray(
        np.stack([_f32(Wc.real), _f32(Wc.imag), _f32(-Wc.imag)]
                 ).reshape(3, 16, 128, 4, 128).transpose(0, 1, 3, 2, 4))
    srb_pack = np.stack([_f32(sr_b.real), _f32(sr_b.imag)]
                        ).reshape(2, 4, 128, 1)

    in_maps = []
    for core in range(8):
        b, g = core // 2, core % 2
        cols = slice(256 * g, 256 * (g + 1))
        Wq_c = Wq[:, cols]
        Wk_c = Wkv_eff[:, :C][:, cols] * SCALE
        Wv_c = Wkv_eff[:, C:][:, cols]
        bk_c = bkv_full[:C][cols] * SCALE
        bv_c = bkv_full[C:][cols]
        u_k = Wk_c.sum(axis=0)
        u_v = Wv_c.sum(axis=0)

        wq_pack = np.empty((4, 2, 4, 128, 128), np.float32)
        wk_pack = np.empty((4, 2, 4, 128, 128), np.float32)
        for h in range(4):
            hc = slice(64 * h, 64 * (h + 1))
            qr = _f32(Wq_c.real[:, hc])
            qi = _f32(Wq_c.imag[:, hc])
            kr = _f32(Wk_c.real[:, hc])
            ki = _f32(Wk_c.imag[:, hc])
            wq_pack[h, 0] = np.concatenate([qr, qi], 1).reshape(4, 128, 128)
            wq_pack[h, 1] = np.concatenate([-qi, qr], 1).reshape(4, 128, 128)
            wk_pack[h, 0] = np.concatenate([kr, -ki], 1).reshape(4, 128, 128)
            wk_pack[h, 1] = np.concatenate([-ki, -kr], 1).reshape(4, 128, 128)

        wv_pack = np.stack([_f32(Wv_c.real), _f32(Wv_c.imag),
                            _f32(-Wv_c.imag)]).reshape(3, 4, 128, 256)
        wp_c = Wproj[256 * g:256 * (g + 1), :]
        wp_pack = np.stack([_f32(wp_c.real), _f32(wp_c.imag),
                            _f32(-wp_c.imag)]).reshape(3, 2, 128, C)

        uk_pack = np.empty((4, 4, 1, 128), np.float32)
        for h in range(4):
            hc = slice(64 * h, 64 * (h + 1))
            ukr, uki = _f32(u_k.real[hc]), _f32(u_k.imag[hc])
            bkr, bki = _f32(bk_c.real[hc]), _f32(bk_c.imag[hc])
            uk_pack[h, 0, 0] = np.concatenate([-ukr, uki])
            uk_pack[h, 1, 0] = np.concatenate([uki, ukr])
            uk_pack[h, 2, 0] = np.concatenate([bkr, -bki])
            uk_pack[h, 3, 0] = np.concatenate([-bki, -bkr])

        uvr, uvi = _f32(u_v.real), _f32(u_v.imag)
        bvr, bvi = _f32(bv_c.real), _f32(bv_c.imag)
        uv_pack = np.stack([-uvr, uvi, -uvi, bvr, -bvi, bvi]
                           ).reshape(6, 1, 256)

        xs_c = np.stack([x_re[b].T, x_im[b].T])  # [2, C, N]
        xsp = xs_c.reshape(2, C, HR, 2, HR, 2)
        xP = np.stack([xsp[:, :, :, p, :, q].reshape(2, C, NK)
                       for p in range(2) for q in range(2)], axis=1)
        m = {
            "xP": _f32(xP.reshape(2, 16, 128, NK)),
            "wc": wc_pack,
            "srb": srb_pack,
            "onesc": np.ones((128, 1), np.float32),
            "wq": wq_pack,
            "wk": wk_pack,
            "wv": wv_pack,
            "wp": wp_pack,
            "uk": uk_pack,
            "uv": uv_pack,
        }
        in_maps.append(m)
    return in_maps


_NC_CACHE = None


def _get_nc():
    global _NC_CACHE
    if _NC_CACHE is None:
        _NC_CACHE = build_nc()
    return _NC_CACHE


def kernel(x_re, x_im, Wq, Wkv, Wproj, bproj, sr_w, sr_b, gain, bias, H, W):
    from concourse.bass_utils import run_bass_kernel_spmd

    nc = _get_nc()
    in_maps = host_prep(x_re, x_im, Wq, Wkv, Wproj, bproj, sr_w, sr_b,
                        gain, bias)
    res = run_bass_kernel_spmd(nc, in_maps, list(range(8)))
    bproj = np.asarray(bproj)
    out = np.zeros((B, N, C), dtype=np.complex64)
    for b in range(B):
        p0 = res.results[2 * b]["outT"]
        p1 = res.results[2 * b + 1]["outT"]
        acc = p0 + p1  # [2, 4cc, 5qc, 128, 512]
        accf = acc.transpose(0, 1, 3, 2, 4).reshape(2, 512, 5 * 512)[:, :, :N]
        full = np.empty((N, C), np.complex64)
        full[_PERM, :] = (accf[0] + 1j * accf[1]).T
        out[b] = full + bproj[None, :]
    return out


# revision 4
# speedup vs baseline: 1.1498x; 1.1498x over previous
"""Trainium2 Bass kernel v2 for complex-valued spatial-reduction attention.

x: [B=4, N=2304, C=512] complex64, H=W=48, 8 heads, hd=64, sr=2 -> Nk=576.
Sharding: 8 cores = 4 batches x 2 head-groups (4 heads each).

v2 restructure vs baseline:
- conv emitted directly in [c_out, nk] layout (no PE transposes/staging).
- complex LayerNorm fully FOLDED: per-token m (mean) and alpha=1/sqrt(var+eps)
  commute through the kv projection: k = alpha*(Wk^T y - m*u) + bk with
  u = colsum(Wk). The -m*u / (1/alpha)*bk terms are rank-1 matmuls into the
  kv PSUM; the complex alpha is applied on DVE (broadcast tiles for k,
  per-partition scalars for v). No elementwise LN over [C, Nk] at all.
- scores: q per head stored [qr;qi] on partitions, k as kA=[kr;-ki],
  kB=[ki;kr]: each score component is ONE K=128 matmul (half the rows).
- softmax chain in f32: custom DVE op SQ2 (sre^2+sim^2 in one pass from two
  PSUM banks), ACT sqrt in-place, ACT exp in-place (bitcast f32r), batched
  per q-chunk so each needs only two ACT table loads.
- denominator: partition-packed PSUM groups; reciprocal on [1,nq] rows;
  broadcast via gpsimd partition_broadcast (no PE broadcast matmuls).
- f32 output partials DMA'd straight from PSUM; host sums core pairs.
"""

import os
import contextlib

import numpy as np

import concourse.bass as bass
import concourse.mybir as mybir
import concourse.tile as tile
from concourse import bacc
from concourse.masks import make_identity

F32 = mybir.dt.float32
F32R = mybir.dt.float32r
AF = mybir.ActivationFunctionType
ALU = mybir.AluOpType

B, N, C, HEADS, HD, SR = 4, 2304, 512, 8, 64, 2
NK = 576
HR = 24
EPS = 1e-5
SCALE = HD ** -0.5  # folded into Wk host-side

K_CHUNKS = [(0, 128), (128, 128), (256, 128), (384, 128), (512, 64)]
Q_CHUNKS = [(0, 512), (512, 512), (1024, 512), (1536, 512), (2048, 256)]
NKH = [(0, 288), (288, 288)]

KDBG = int(os.environ.get("KDBG", "0"))
KSQ2 = int(os.environ.get("KSQ2", "1"))   # 1: custom DVE sq2 op
KPBC = int(os.environ.get("KPBC", "1"))   # 1: gpsimd partition_broadcast


# ---------------------------------------------------------------------------
# custom DVE op: out = in0^2 + in1^2 (one pass; both inputs may be PSUM)
# ---------------------------------------------------------------------------
_SQ2 = None


def _get_sq2():
    global _SQ2
    if _SQ2 is not None:
        return _SQ2
    import re
    from concourse import dve_ops
    from concourse.dve_spec import Spec, Src0, Src1, sq

    for o in dve_ops.OPS:
        if o.name == "SQ2_ANT":
            _SQ2 = o
            return o
    op = dve_ops.DveOp(
        "SQ2_ANT",
        Spec(body=sq(Src0) + sq(Src1),
             reference=lambda in0, in1, s0, s1, imm2: (
                 np.asarray(in0, np.float32) ** 2
                 + np.asarray(in1, np.float32) ** 2)),
        subdim=False, uops_sha={})
    dve_ops.OPS.append(op)
    dve_ops.CUSTOM_DVE_SPECS[op.name] = op.spec
    dve_ops._SUB_OPCODE_FOR_NAME[op.name] = (
        max(dve_ops._SUB_OPCODE_FOR_NAME.values()) + 1)
    for ver in ("v3", "v4"):
        try:
            op.compile(ver)
        except ValueError as e:
            m = re.search(r"(v\d): ([0-9a-f]{16})", str(e))
            op.uops_sha[m.group(1)] = m.group(2)
            op.compile(ver)
    _SQ2 = op
    return op


def build_nc():
    nc = bacc.Bacc("TRN2", target_bir_lowering=False, debug=False,
                   num_devices=8)

    xP_d = nc.dram_tensor("xP", [2, 16, 128, NK], F32R, kind="ExternalInput")
    wc_d = nc.dram_tensor("wc", [3, 16, 4, 128, 128], F32R, kind="ExternalInput")
    srb_d = nc.dram_tensor("srb", [2, 4, 128, 1], F32, kind="ExternalInput")
    onesc_d = nc.dram_tensor("onesc", [128, 1], F32R, kind="ExternalInput")
    wq_d = nc.dram_tensor("wq", [4, 2, 4, 128, 128], F32R, kind="ExternalInput")
    wk_d = nc.dram_tensor("wk", [4, 2, 4, 128, 128], F32R, kind="ExternalInput")
    wv_d = nc.dram_tensor("wv", [3, 4, 128, 256], F32R, kind="ExternalInput")
    wp_d = nc.dram_tensor("wp", [3, 2, 128, 512], F32R, kind="ExternalInput")
    uk_d = nc.dram_tensor("uk", [4, 4, 1, 128], F32R, kind="ExternalInput")
    uv_d = nc.dram_tensor("uv", [6, 1, 256], F32R, kind="ExternalInput")
    outT_d = nc.dram_tensor("outT", [2, 4, 5, 128, 512], F32,
                            kind="ExternalOutput")
    dbg = {}
    if KDBG:
        dbg["xconv"] = nc.dram_tensor("dbg_xconv", [2, C, NK], F32,
                                      kind="ExternalOutput")
        dbg["alpha"] = nc.dram_tensor("dbg_alpha", [6, NK], F32,
                                      kind="ExternalOutput")
        dbg["kAB"] = nc.dram_tensor("dbg_kAB", [2, 4, 128, NK], F32,
                                    kind="ExternalOutput")
        dbg["vpk"] = nc.dram_tensor("dbg_vpk", [128, 5 * 4 * 128], F32,
                                    kind="ExternalOutput")
        dbg["q"] = nc.dram_tensor("dbg_q", [4, 128, N], F32,
                                  kind="ExternalOutput")

    with tile.TileContext(nc) as tc:
        _body(nc, tc, xP_d, wc_d, srb_d, onesc_d, wq_d, wk_d, wv_d, wp_d,
              uk_d, uv_d, outT_d, dbg)

    nc.compile()
    # the stock walrus birverifier asserts on custom ISA rows (SQ2_ANT,
    # PartitionBroadcast); codegen emits the pre-encoded bytes fine, so
    # skip verification for those instructions.
    for f in nc.m.functions:
        for blk in f.blocks:
            for inst in blk.instructions:
                if isinstance(inst, mybir.InstISA) and inst.verify:
                    inst.verify = False
    return nc


def _sqrt_newton(nc, pool, out, x, sz, sc, tagp):
    """out = sqrt(sc*max(x,0)), ACT LUT seed + one Newton step."""
    nc.vector.tensor_scalar_max(x[:, :sz], x[:, :sz], 0.0)
    y0 = pool.tile([128, 8], F32, tag=f"{tagp}_y0", name=f"{tagp}_y0")
    nc.scalar.activation(y0[:, :sz], x[:, :sz], AF.Sqrt, scale=sc)
    yr = pool.tile([128, 8], F32, tag=f"{tagp}_yr", name=f"{tagp}_yr")
    nc.vector.tensor_scalar_max(y0[:, :sz], y0[:, :sz], 1e-12)
    nc.vector.reciprocal_approx_fast(yr[:, :sz], y0[:, :sz])
    nc.vector.tensor_mul(yr[:, :sz], yr[:, :sz], x[:, :sz])
    if sc != 1.0:
        nc.vector.tensor_scalar_mul(yr[:, :sz], yr[:, :sz], sc)
    nc.vector.tensor_add(out[:, :sz], y0[:, :sz], yr[:, :sz])
    nc.vector.tensor_scalar_mul(out[:, :sz], out[:, :sz], 0.5)


def _body(nc, tc, xP_d, wc_d, srb_d, onesc_d, wq_d, wk_d, wv_d, wp_d,
          uk_d, uv_d, outT_d, dbg):
    SQ2 = _get_sq2() if KSQ2 else None
    ctx = contextlib.ExitStack()
    consts = ctx.enter_context(tc.tile_pool(name="consts", bufs=1))
    big = ctx.enter_context(tc.tile_pool(name="big", bufs=1))

    ident = consts.tile([128, 128], F32, tag="ident", name="ident")
    make_identity(nc, ident)
    onesc = consts.tile([128, 1], F32R, tag="onesc", name="onesc")
    nc.sync.dma_start(onesc[:], onesc_d[:, :])
    nbias = consts.tile([128, 1], F32, tag="nbias", name="nbias")
    nc.vector.memset(nbias, -50.0)
    onesr_c = consts.tile([1, 128], F32R, tag="onesrc", name="onesr_c")
    nc.vector.memset(onesr_c.bitcast(F32), 1.0)
    srb_sb = [[consts.tile([128, 1], F32, tag=f"srb{p}{t}",
                           name=f"srb{p}{t}") for t in range(4)]
              for p in range(2)]
    for p in range(2):
        for t in range(4):
            nc.sync.dma_start(srb_sb[p][t][:], srb_d[p, t])

    # whole-kernel SBUF residents (kept minimal: SBUF is tight in phase A)
    q_sb = big.tile([128, 4, N], F32R, tag="q_sb", name="q_sb")
    aw = big.tile([128, 5, 16], F32, tag="aw", name="aw")

    # =====================================================================
    # Phase A: conv in [c_out, nk] layout + q-projection interleaved
    # =====================================================================
    if True:
        bigA = ctx.enter_context(tc.tile_pool(name="bigA", bufs=1))
        xconv = bigA.tile([128, 2, 4, NK], F32R, tag="xconv", name="xconv")
        stats_part = bigA.tile([1, 5, NK], F32, tag="spart", name="spart")

        with tc.tile_pool(name="xpA", bufs=1) as xpA, \
             tc.tile_pool(name="cwork", bufs=2) as cwork, \
             tc.tile_pool(name="sqp", bufs=1) as sqp, \
             tc.tile_pool(name="cvps", bufs=4, space="PSUM") as cvps, \
             tc.tile_pool(name="qpps", bufs=2, space="PSUM") as qpps, \
             tc.tile_pool(name="stps", bufs=2, space="PSUM") as stps:

            xP = xpA.tile([128, 2, 16, NK], F32R, tag="xP", name="xP")
            wq = xpA.tile([128, 4, 2, 4, 128], F32R, tag="wq", name="wq")
            for h in range(4):
                for mv in range(2):
                    for cj in range(4):
                        nc.gpsimd.dma_start(wq[:, h, mv, cj, :],
                                            wq_d[h, mv, cj])

            def emit_q(p4):
                for h in range(4):
                    prs = []
                    for ni in range(2):
                        prs.append(qpps.tile([128, 288], F32, tag="qp",
                                             name=f"q{p4}_{h}_{ni}"))
                    for mv in range(2):
                        for cj in range(4):
                            st = mv == 0 and cj == 0
                            sp = mv == 1 and cj == 3
                            for ni, (n0, nn) in enumerate(NKH):
                                nc.tensor.matmul(
                                    prs[ni][:, :nn], wq[:, h, mv, cj, :],
                                    xP[:, mv, 4 * p4 + cj, n0:n0 + nn],
                                    start=st, stop=sp)
                    with nc.allow_low_precision(reason="f32r rounding"):
                        for ni, (n0, nn) in enumerate(NKH):
                            q0 = p4 * NK + n0
                            nc.vector.tensor_copy(
                                q_sb[:, h, q0:q0 + nn], prs[ni][:, :nn])

            for t in range(4):
                cps = []
                for pl in range(2):
                    for ni in range(2):
                        cps.append(cvps.tile([128, 288], F32, tag="cv",
                                             name=f"cv{t}_{pl}{ni}"))
                cs = slice(128 * t, 128 * (t + 1))
                for kk in range(16):
                    if t == 0:
                        nc.sync.dma_start(xP[:, 0, kk, :], xP_d[0, kk])
                        nc.gpsimd.dma_start(xP[:, 1, kk, :], xP_d[1, kk])
                    wcr = cwork.tile([128, 128], F32R, tag="wc_r", name="wcr")
                    wci = cwork.tile([128, 128], F32R, tag="wc_i", name="wci")
                    wcm = cwork.tile([128, 128], F32R, tag="wc_m", name="wcm")
                    nc.sync.dma_start(wcr[:], wc_d[0, kk, t])
                    nc.gpsimd.dma_start(wci[:], wc_d[1, kk, t])
                    nc.sync.dma_start(wcm[:], wc_d[2, kk, t])
                    st = kk == 0
                    sp = kk == 15
                    for ni, (n0, nn) in enumerate(NKH):
                        xr = xP[:, 0, kk, n0:n0 + nn]
                        xi = xP[:, 1, kk, n0:n0 + nn]
                        cr = cps[ni]
                        ci = cps[2 + ni]
                        nc.tensor.matmul(cr[:, :nn], wcr[:], xr, start=st,
                                         stop=False)
                        nc.tensor.matmul(ci[:, :nn], wcr[:], xi, start=st,
                                         stop=False)
                        nc.tensor.matmul(ci[:, :nn], wci[:], xr, start=False,
                                         stop=sp)
                        nc.tensor.matmul(cr[:, :nn], wcm[:], xi, start=False,
                                         stop=sp)
                    if t == 0 and kk == 7:
                        emit_q(0)
                # epilogue: copy + sr_b bias into SBUF, squares, stat partials
                with nc.allow_low_precision(reason="f32r rounding modeled"):
                    for pl in range(2):
                        for ni, (n0, nn) in enumerate(NKH):
                            nc.vector.tensor_scalar_add(
                                xconv[:, pl, t, n0:n0 + nn],
                                cps[2 * pl + ni][:, :nn], srb_sb[pl][t][:])
                    sq = sqp.tile([128, 3, NK], F32R, tag="sq",
                                  name=f"sq{t}")
                    nc.scalar.activation(sq[:, 0, :], xconv[:, 0, t, :],
                                         AF.Square)
                    nc.scalar.activation(sq[:, 1, :], xconv[:, 1, t, :],
                                         AF.Square)
                    nc.vector.tensor_mul(sq[:, 2, :], xconv[:, 0, t, :],
                                         xconv[:, 1, t, :])
                movs = [xconv[:, 0, t, :],
                        xconv[:, 1, t, :],
                        sq[:, 0, :],
                        sq[:, 1, :],
                        sq[:, 2, :]]
                for qy in range(5):
                    for ni, (n0, nn) in enumerate(NKH):
                        sps = stps.tile([128, 288], F32, tag="st",
                                        name=f"st{t}_{qy}_{ni}")
                        nc.tensor.matmul(sps[0:1, :nn], onesc[:],
                                         movs[qy][:, n0:n0 + nn],
                                         start=True, stop=True)
                        if t == 0:
                            nc.vector.tensor_copy(
                                stats_part[:, qy, n0:n0 + nn],
                                sps[0:1, :nn])
                        else:
                            nc.vector.tensor_add(
                                stats_part[:, qy, n0:n0 + nn],
                                stats_part[:, qy, n0:n0 + nn],
                                sps[0:1, :nn])
                if t < 2:
                    emit_q(t + 1)
            emit_q(3)

        # ---- kv-weight DMAs (overlap the alpha chain) ----
        bigBC = ctx.enter_context(tc.tile_pool(name="bigBC", bufs=1))
        kAf = bigBC.tile([128, 4, NK], F32R, tag="kAf", name="kAf")
        kBf = bigBC.tile([128, 4, NK], F32R, tag="kBf", name="kBf")
        vpk = bigBC.tile([128, 5, 4, 128], F32R, tag="vpk", name="vpk")
        wp = bigBC.tile([128, 3, 2, 512], F32R, tag="wp", name="wp")
        uk = bigBC.tile([1, 4, 4, 128], F32R, tag="uk", name="uk")
        uv = bigBC.tile([1, 6, 256], F32R, tag="uv", name="uv")
        with tc.tile_pool(name="poolB", bufs=1) as poolB, \
             tc.tile_pool(name="tpps", bufs=2, space="PSUM") as tpps:
            wk = poolB.tile([128, 4, 2, 4, 128], F32R, tag="wk", name="wk")
            wv = poolB.tile([128, 3, 4, 256], F32R, tag="wv", name="wv")
            arb = poolB.tile([128, 2, NK], F32R, tag="arb", name="arb")
            arow_all = poolB.tile([1, 6, NK], F32R, tag="arows",
                                  name="arow_all")
            for h in range(4):
                for mv in range(2):
                    for cj in range(4):
                        eng = nc.sync if cj % 2 == 0 else nc.gpsimd
                        eng.dma_start(wk[:, h, mv, cj, :], wk_d[h, mv, cj])
            for pl in range(3):
                for cj in range(4):
                    nc.gpsimd.dma_start(wv[:, pl, cj, :], wv_d[pl, cj])
                for hp in range(2):
                    nc.sync.dma_start(wp[:, pl, hp, :], wp_d[pl, hp])
            for h in range(4):
                for j in range(4):
                    nc.sync.dma_start(uk[:, h, j, :], uk_d[h, j])
            for j in range(6):
                nc.gpsimd.dma_start(uv[:, j, :], uv_d[j])

            stats_sb = stats_part

            # ---- alpha/m math on token-partitions ----
            for c5 in range(5):
                k0, szk = K_CHUNKS[c5]
                tp = tpps.tile([128, 128], F32, tag="tp", name=f"tp{c5}")
                for qy in range(5):
                    nc.tensor.transpose(tp[:szk, qy:qy + 1],
                                        stats_sb[:, qy, k0:k0 + szk],
                                        ident[:1, :1])
                nc.vector.tensor_copy(aw[:szk, c5, 0:5], tp[:szk, 0:5])

            inv_c = 1.0 / C
            A = lambda q: aw[:, :, q]
            awp = lambda q: aw[:, :, q:q + 1].rearrange("p a b -> p (a b)")
            # slots: 0..4 sums; 5=mr 6=mi 7=sr 8=si 9=ar 10=ai
            # 11,12 scratch; 13=vre 14=vim 15=|v|
            nc.vector.tensor_scalar_mul(A(5), A(0), inv_c)
            nc.vector.tensor_scalar_mul(A(6), A(1), inv_c)
            nc.vector.tensor_sub(A(13), A(2), A(3))
            nc.vector.tensor_scalar_mul(A(13), A(13), inv_c)
            nc.vector.tensor_mul(A(11), A(5), A(5))
            nc.vector.tensor_mul(A(12), A(6), A(6))
            nc.vector.tensor_sub(A(11), A(11), A(12))
            nc.vector.tensor_sub(A(13), A(13), A(11))
            nc.vector.tensor_scalar_add(A(13), A(13), EPS)   # vre
            nc.vector.tensor_mul(A(11), A(5), A(6))
            nc.vector.tensor_scalar_mul(A(11), A(11), 2.0)
            nc.vector.tensor_scalar_mul(A(14), A(4), 2.0 * inv_c)
            nc.vector.tensor_sub(A(14), A(14), A(11))        # vim
            nc.vector.tensor_mul(A(11), A(13), A(13))
            nc.vector.tensor_mul(A(12), A(14), A(14))
            nc.vector.tensor_add(A(11), A(11), A(12))        # |v|^2
            _sqrt_newton(nc, consts, awp(15), awp(11), 5, 1.0, "nv")
            nc.vector.tensor_add(A(11), A(15), A(13))
            _sqrt_newton(nc, consts, awp(7), awp(11), 5, 0.5, "nr")
            nc.vector.tensor_sub(A(11), A(15), A(13))
            _sqrt_newton(nc, consts, awp(8), awp(11), 5, 0.5, "ni")
            sgn = consts.tile([128, 8], F32, tag="sgn", name="sgn")
            nc.scalar.activation(sgn[:, 0:5], A(14), AF.Sign)
            nc.vector.tensor_mul(A(8), A(8), sgn[:, 0:5])             # si
            rin = consts.tile([128, 8], F32, tag="rin", name="rin")
            nc.vector.reciprocal_approx_fast(rin[:, 0:5], A(15))
            nc.vector.tensor_mul(A(9), A(7), rin[:, 0:5])             # ar
            nc.vector.tensor_mul(A(10), A(8), rin[:, 0:5])
            nc.vector.tensor_scalar_mul(A(10), A(10), -1.0)           # ai

            # transpose back -> arow_all rows (mr mi sr si ar ai)
            for c5 in range(5):
                k0, szk = K_CHUNKS[c5]
                for j in range(6):
                    tpb = tpps.tile([128, 128], F32, tag="tp",
                                    name=f"tpb{c5}_{j}")
                    nc.tensor.transpose(tpb[0:1, :szk],
                                        aw[:szk, c5, 5 + j:6 + j],
                                        ident[:szk, :szk])
                    with nc.allow_low_precision(reason="f32r rounding"):
                        nc.vector.tensor_copy(arow_all[:, j, k0:k0 + szk],
                                              tpb[0:1, :szk])
            if KPBC:
                nc.gpsimd.partition_broadcast(arb[:, 0, :],
                                              arow_all[:, 4, :])
                nc.gpsimd.partition_broadcast(arb[:, 1, :],
                                              arow_all[:, 5, :])
            else:
                onesr = consts.tile([1, 128], F32R, tag="onesr",
                                    name="onesr")
                nc.vector.memset(onesr.bitcast(F32), 1.0)
                for j, (a0, aa) in enumerate(((4, 0), (5, 1))):
                    for n0, nn in ((0, 512), (512, 64)):
                        bp = tpps.tile([128, 512], F32, tag="bp",
                                       name=f"bp{j}")
                        nc.tensor.matmul(bp[:, :nn], onesr[:],
                                         arow_all[:, a0, n0:n0 + nn],
                                         start=True, stop=True)
                        nc.vector.tensor_copy(arb[:, aa, n0:n0 + nn],
                                              bp[:, :nn])

            # =============================================================
            # Phase B: k (kA/kB + rank-1 folds + alpha), v (+alpha)
            # =============================================================
            with tc.tile_pool(name="kvps", bufs=2, space="PSUM") as kvps, \
                 tc.tile_pool(name="vps", bufs=4 if KPBC else 2, space="PSUM") as vps, \
                 tc.tile_pool(name="kw", bufs=1) as kw:
                ar_b = arb.bitcast(F32)[:, 0, :]
                ai_b = arb.bitcast(F32)[:, 1, :]
                arr = lambda j: arow_all[:, j, :]
                for h in range(4):
                    kps = []
                    for ni in range(2):
                        kps.append(kvps.tile([128, 288], F32, tag="kp",
                                             name=f"k{h}_{ni}"))
                    for mv in range(2):
                        for cj in range(4):
                            for ni, (n0, nn) in enumerate(NKH):
                                nc.tensor.matmul(
                                    kps[ni][:, :nn], wk[:, h, mv, cj, :],
                                    xconv[:, mv, cj,
                                                        n0:n0 + nn],
                                    start=(mv == 0 and cj == 0), stop=False)
                    for j in range(4):
                        for ni, (n0, nn) in enumerate(NKH):
                            nc.tensor.matmul(
                                kps[ni][:, :nn], uk[:, h, j, :],
                                arr(j)[:, n0:n0 + nn],
                                start=False, stop=j == 3)
                    for ni, (n0, nn) in enumerate(NKH):
                        kat = kw.tile([128, 288], F32, tag="kat",
                                      name=f"kat{h}_{ni}")
                        kbt = kw.tile([128, 288], F32, tag="kbt",
                                      name=f"kbt{h}_{ni}")
                        t1 = kw.tile([128, 288], F32, tag="kt1",
                                     name=f"kt1_{h}_{ni}")
                        t2 = kw.tile([128, 288], F32, tag="kt2",
                                     name=f"kt2_{h}_{ni}")
                        ns = slice(n0, n0 + nn)
                        ab_r = arb.bitcast(F32)[:, 0, ns]
                        ab_i = arb.bitcast(F32)[:, 1, ns]
                        nc.vector.tensor_copy(kat[:, :nn], kps[ni][:, :nn])
                        nc.vector.tensor_scalar_mul(kbt[0:64, :nn],
                                                    kat[64:128, :nn], -1.0)
                        nc.vector.tensor_copy(kbt[64:128, :nn],
                                              kat[0:64, :nn])
                        nc.vector.tensor_mul(t1[:, :nn], ab_r, kat[:, :nn])
                        nc.vector.tensor_mul(t2[:, :nn], ab_i, kbt[:, :nn])
                        with nc.allow_low_precision(reason="f32r rounding"):
                            nc.vector.tensor_sub(kAf[:, h, ns], t1[:, :nn],
                                                 t2[:, :nn])
                        nc.vector.tensor_mul(t1[:, :nn], ab_r, kbt[:, :nn])
                        nc.vector.tensor_mul(t2[:, :nn], ab_i, kat[:, :nn])
                        with nc.allow_low_precision(reason="f32r rounding"):
                            nc.vector.tensor_add(kBf[:, h, ns], t1[:, :nn],
                                                 t2[:, :nn])

                # ---- v ----
                stt = nc.vector.scalar_tensor_tensor
                for kc, (k0, szk) in enumerate(K_CHUNKS):
                    vr = vps.tile([128, 256], F32, tag="vp", name=f"vr{kc}")
                    vi = vps.tile([128, 256], F32, tag="vp", name=f"vi{kc}")
                    for cj in range(4):
                        xr = xconv[:, 0, cj, k0:k0 + szk]
                        xi = xconv[:, 1, cj, k0:k0 + szk]
                        st = cj == 0
                        nc.tensor.matmul(vr[:szk, :], xr, wv[:, 0, cj, :],
                                         start=st, stop=False)
                        nc.tensor.matmul(vi[:szk, :], xr, wv[:, 1, cj, :],
                                         start=st, stop=False)
                        nc.tensor.matmul(vr[:szk, :], xi, wv[:, 2, cj, :],
                                         start=False, stop=False)
                        nc.tensor.matmul(vi[:szk, :], xi, wv[:, 0, cj, :],
                                         start=False, stop=False)
                    vr_terms = [(0, 0), (1, 1), (2, 3), (3, 4)]
                    vi_terms = [(0, 2), (1, 0), (2, 5), (3, 3)]
                    for i, (arow, uvrow) in enumerate(vr_terms):
                        nc.tensor.matmul(vr[:szk, :],
                                         arr(arow)[:, k0:k0 + szk],
                                         uv[:, uvrow, :], start=False,
                                         stop=i == 3)
                    for i, (arow, uvrow) in enumerate(vi_terms):
                        nc.tensor.matmul(vi[:szk, :],
                                         arr(arow)[:, k0:k0 + szk],
                                         uv[:, uvrow, :], start=False,
                                         stop=i == 3)
                    # alpha apply (per-partition scalars from aw chunk kc)
                    tmr = kw.tile([128, 256], F32, tag="tmr", name=f"tmr{kc}")
                    tmi = kw.tile([128, 256], F32, tag="tmi", name=f"tmi{kc}")
                    ar_s = awp(9)[:, kc:kc + 1]
                    ai_s = awp(10)[:, kc:kc + 1]
                    nc.vector.tensor_scalar_mul(tmr[:szk], vi[:szk, :],
                                                ai_s[:szk])
                    nc.vector.tensor_scalar_mul(tmi[:szk], vr[:szk, :],
                                                ai_s[:szk])
                    vout_r = vpk[:szk, kc, :, 0:64]
                    vout_i = vpk[:szk, kc, :, 64:128]
                    vrr = vr[:szk, :].rearrange("p (h d) -> p h d", h=4)
                    vir = vi[:szk, :].rearrange("p (h d) -> p h d", h=4)
                    tmrr = tmr[:szk].rearrange("p (h d) -> p h d", h=4)
                    tmir = tmi[:szk].rearrange("p (h d) -> p h d", h=4)
                    with nc.allow_low_precision(reason="f32r rounding"):
                        stt(vout_r, vrr, ar_s[:szk], tmrr, ALU.mult,
                            ALU.subtract)
                        stt(vout_i, vir, ar_s[:szk], tmir, ALU.mult, ALU.add)

                if KDBG:
                    for pl in range(2):
                        for t in range(4):
                            nc.sync.dma_start(
                                dbg["xconv"][pl, 128 * t:128 * (t + 1), :],
                                xconv.bitcast(F32)[:, pl, t, :])
                    for j in range(6):
                        nc.sync.dma_start(dbg["alpha"][j:j + 1, :],
                                          arow_all.bitcast(F32)[:, j, :])
                    for h in range(4):
                        nc.sync.dma_start(dbg["kAB"][0, h],
                                          kAf.bitcast(F32)[:, h, :])
                        nc.sync.dma_start(dbg["kAB"][1, h],
                                          kBf.bitcast(F32)[:, h, :])
                        nc.sync.dma_start(dbg["q"][h],
                                          q_sb.bitcast(F32)[:, h, :])
                    nc.sync.dma_start(
                        dbg["vpk"][:, :],
                        vpk.bitcast(F32).rearrange("p a b c -> p (a b c)"))

    # =====================================================================
    # Phase C: attention + projection, software-pipelined over q-chunks
    # =====================================================================
    with tc.tile_pool(name="sm", bufs=1) as sm, \
         tc.tile_pool(name="scps", bufs=4, space="PSUM") as scps, \
         tc.tile_pool(name="ovps", bufs=1, space="PSUM") as ovps, \
         tc.tile_pool(name="dnps", bufs=1, space="PSUM") as dnps, \
         tc.tile_pool(name="pjps", bufs=2, space="PSUM") as pjps:

        def emit_front_h(qi, q0, nq, h, stiles):
            sh = sm.tile([128, 5, 512], F32R, tag="s", bufs=5, name=f"s{h}")
            stiles[h] = sh
            for kc, (k0, szk) in enumerate(K_CHUNKS):
                sre = scps.tile([128, 512], F32, tag="sc",
                                name=f"sre{h}_{kc}")
                sim = scps.tile([128, 512], F32, tag="sc",
                                name=f"sim{h}_{kc}")
                nc.tensor.matmul(sre[:szk, :nq], kAf[:, h, k0:k0 + szk],
                                 q_sb[:, h, q0:q0 + nq], start=True,
                                 stop=True)
                nc.tensor.matmul(sim[:szk, :nq], kBf[:, h, k0:k0 + szk],
                                 q_sb[:, h, q0:q0 + nq], start=True,
                                 stop=True)
                t2 = sm.tile([128, 512], F32, tag="t2", bufs=2, name="t2")
                nc.vector.tensor_copy(t2[:szk, :nq], sim[:szk, :nq])
                with nc.allow_low_precision(reason="pre-sqrt"):
                    nc.vector._custom_dve(SQ2, out=sh[:szk, kc, :nq],
                                          in0=sre[:szk, :nq],
                                          in1=t2[:szk, :nq])

        def emit_act(qi, nq, stiles):
            # |a| = exp(0.5*ln(s)); ebuf = exp(|a| - 50): ln+exp share one
            # ACT table set (natural_log_exp_and_others).
            for h in range(4):
                sh = stiles[h]
                nc.scalar.activation(sh[:, :, :nq], sh[:, :, :nq], AF.Ln)
            for h in range(4):
                sh = stiles[h]
                nc.scalar.activation(sh[:, :, :nq], sh[:, :, :nq], AF.Exp,
                                     scale=0.5)
            for h in range(4):
                sh = stiles[h]
                nc.scalar.activation(sh[:, :, :nq], sh[:, :, :nq], AF.Exp,
                                     bias=nbias[:])

        ot_store = {}

        def emit_back_h(qi, q0, nq, stiles, h):
            ov = ovps.tile([128, 512], F32, tag="ov", name=f"ov{h}")
            dn_ps = dnps.tile([128, 512], F32, tag="dn", name=f"dn{h}")
            sh = stiles[h]
            for kc, (k0, szk) in enumerate(K_CHUNKS):
                eb = sh[:, kc, :]
                nc.tensor.matmul(ov[:, :nq], vpk[:szk, kc, h, :],
                                 eb[:szk, :nq], start=kc == 0,
                                 stop=kc == 4)
                nc.tensor.matmul(dn_ps[0:1, :nq], onesc[:szk],
                                 eb[:szk, :nq], start=kc == 0,
                                 stop=kc == 4)
            dnr = sm.tile([1, 512], F32, tag="dnr", bufs=2, name=f"dnr{h}")
            nc.scalar.copy(dnr[:, :nq], dn_ps[0:1, :nq])
            dri = sm.tile([1, 512], F32, tag="dri", bufs=2, name=f"dri{h}")
            nc.vector.reciprocal_approx_fast(dri[:, :nq], dnr[:, :nq])
            rb = sm.tile([128, 512], F32, tag="rb", bufs=2, name=f"rb{h}")
            if KPBC:
                nc.gpsimd.partition_broadcast(rb[:, :nq], dri[:, :nq])
            else:
                rbp = ovps.tile([128, 512], F32, tag="ov", name=f"rbp{h}")
                nc.tensor.matmul(rbp[:, :nq], onesr_c[:],
                                 dri.bitcast(F32R)[:, :nq],
                                 start=True, stop=True)
                nc.vector.tensor_copy(rb[:, :nq], rbp[:, :nq])
            hp, hi = h // 2, h % 2
            if hi == 0:
                ot_store[(qi, hp)] = (
                    sm.tile([128, 512], F32R, tag="otr", bufs=3,
                            name=f"otr{hp}"),
                    sm.tile([128, 512], F32R, tag="oti", bufs=3,
                            name=f"oti{hp}"))
            otr, oti = ot_store[(qi, hp)]
            rs = slice(64 * hi, 64 * (hi + 1))
            with nc.allow_low_precision(reason="f32r rounding"):
                nc.vector.tensor_mul(otr[rs, :nq], ov[0:64, :nq],
                                     rb[0:64, :nq])
                nc.vector.tensor_mul(oti[rs, :nq], ov[64:128, :nq],
                                     rb[64:128, :nq])

        def emit_proj(qi, q0, nq):
            for cc in range(4):
                cs = slice(128 * cc, 128 * (cc + 1))
                pr = pjps.tile([128, 512], F32, tag="pj", name=f"pr{cc}")
                pi = pjps.tile([128, 512], F32, tag="pj", name=f"pi{cc}")
                for hp in range(2):
                    otr, oti = ot_store[(qi, hp)]
                    st = hp == 0
                    sp = hp == 1
                    nc.tensor.matmul(pr[:, :nq], wp[:, 0, hp, cs],
                                     otr[:, :nq], start=st, stop=False)
                    nc.tensor.matmul(pi[:, :nq], wp[:, 0, hp, cs],
                                     oti[:, :nq], start=st, stop=False)
                    nc.tensor.matmul(pr[:, :nq], wp[:, 2, hp, cs],
                                     oti[:, :nq], start=False, stop=sp)
                    nc.tensor.matmul(pi[:, :nq], wp[:, 1, hp, cs],
                                     otr[:, :nq], start=False, stop=sp)
                o1 = sm.tile([128, 512], F32, tag="o1", bufs=2,
                             name="o1")
                o2 = sm.tile([128, 512], F32, tag="o2", bufs=2,
                             name="o2")
                nc.scalar.copy(o1[:, :nq], pr[:, :nq])
                nc.scalar.copy(o2[:, :nq], pi[:, :nq])
                eng = nc.sync if cc % 2 == 0 else nc.gpsimd
                eng2 = nc.gpsimd if cc % 2 == 0 else nc.sync
                eng.dma_start(outT_d[0, cc, qi, :, :nq], o1[:, :nq])
                eng2.dma_start(outT_d[1, cc, qi, :, :nq], o2[:, :nq])
            del ot_store[(qi, 0)]
            del ot_store[(qi, 1)]

        prev = None
        for qi, (q0, nq) in enumerate(Q_CHUNKS):
            stiles = {}
            for h in range(4):
                emit_front_h(qi, q0, nq, h, stiles)
                if prev is not None:
                    pqi, pq0, pnq, pst = prev
                    emit_back_h(pqi, pq0, pnq, pst, h)
            emit_act(qi, nq, stiles)
            if prev is not None:
                emit_proj(pqi, pq0, pnq)
            prev = (qi, q0, nq, stiles)
        pqi, pq0, pnq, pst = prev
        for h in range(4):
            emit_back_h(pqi, pq0, pnq, pst, h)
        emit_proj(pqi, pq0, pnq)

    ctx.close()


# =========================================================================
# Host side
# =========================================================================

def _f32(x):
    return np.ascontiguousarray(x, dtype=np.float32)


def _perm():
    perm = np.empty(4 * NK, dtype=np.int64)
    for p4 in range(4):
        p, q = p4 // 2, p4 % 2
        for nk in range(NK):
            hi, wi = nk // HR, nk % HR
            perm[p4 * NK + nk] = (SR * hi + p) * (SR * HR) + SR * wi + q
    return perm


_PERM = _perm()


def host_prep(x_re, x_im, Wq, Wkv, Wproj, bproj, sr_w, sr_b, gain, bias):
    x_re = np.asarray(x_re)
    x_im = np.asarray(x_im)
    Wq = np.asarray(Wq)
    Wkv = np.asarray(Wkv)
    Wproj = np.asarray(Wproj)
    sr_w = np.asarray(sr_w)
    sr_b = np.asarray(sr_b)
    gain = np.asarray(gain)
    bias = np.asarray(bias)

    Wkv_eff = gain[:, None] * Wkv
    bkv_full = bias @ Wkv
    Wc = sr_w.transpose(2, 3, 1, 0).reshape(4 * C, C)
    wc_pack = np.ascontiguousarray(
        np.stack([_f32(Wc.real), _f32(Wc.imag), _f32(-Wc.imag)]
                 ).reshape(3, 16, 128, 4, 128).transpose(0, 1, 3, 2, 4))
    srb_pack = np.stack([_f32(sr_b.real), _f32(sr_b.imag)]
                        ).reshape(2, 4, 128, 1)

    in_maps = []
    for core in range(8):
        b, g = core // 2, core % 2
        cols = slice(256 * g, 256 * (g + 1))
        Wq_c = Wq[:, cols]
        Wk_c = Wkv_eff[:, :C][:, cols] * SCALE
        Wv_c = Wkv_eff[:, C:][:, cols]
        bk_c = bkv_full[:C][cols] * SCALE
        bv_c = bkv_full[C:][cols]
        u_k = Wk_c.sum(axis=0)
        u_v = Wv_c.sum(axis=0)

        wq_pack = np.empty((4, 2, 4, 128, 128), np.float32)
        wk_pack = np.empty((4, 2, 4, 128, 128), np.float32)
        for h in range(4):
            hc = slice(64 * h, 64 * (h + 1))
            qr = _f32(Wq_c.real[:, hc])
            qi = _f32(Wq_c.imag[:, hc])
            kr = _f32(Wk_c.real[:, hc])
            ki = _f32(Wk_c.imag[:, hc])
            wq_pack[h, 0] = np.concatenate([qr, qi], 1).reshape(4, 128, 128)
            wq_pack[h, 1] = np.concatenate([-qi, qr], 1).reshape(4, 128, 128)
            wk_pack[h, 0] = np.concatenate([kr, -ki], 1).reshape(4, 128, 128)
            wk_pack[h, 1] = np.concatenate([-ki, -kr], 1).reshape(4, 128, 128)

        wv_pack = np.stack([_f32(Wv_c.real), _f32(Wv_c.imag),
                            _f32(-Wv_c.imag)]).reshape(3, 4, 128, 256)
        wp_c = Wproj[256 * g:256 * (g + 1), :]
        wp_pack = np.stack([_f32(wp_c.real), _f32(wp_c.imag),
                            _f32(-wp_c.imag)]).reshape(3, 2, 128, C)

        uk_pack = np.empty((4, 4, 1, 128), np.float32)
        for h in range(4):
            hc = slice(64 * h, 64 * (h + 1))
            ukr, uki = _f32(u_k.real[hc]), _f32(u_k.imag[hc])
            bkr, bki = _f32(bk_c.real[hc]), _f32(bk_c.imag[hc])
            uk_pack[h, 0, 0] = np.concatenate([-ukr, uki])
            uk_pack[h, 1, 0] = np.concatenate([uki, ukr])
            uk_pack[h, 2, 0] = np.concatenate([bkr, -bki])
            uk_pack[h, 3, 0] = np.concatenate([-bki, -bkr])

        uvr, uvi = _f32(u_v.real), _f32(u_v.imag)
        bvr, bvi = _f32(bv_c.real), _f32(bv_c.imag)
        uv_pack = np.stack([-uvr, uvi, -uvi, bvr, -bvi, bvi]
                           ).reshape(6, 1, 256)

        xs_c = np.stack([x_re[b].T, x_im[b].T])  # [2, C, N]
        xsp = xs_c.reshape(2, C, HR, 2, HR, 2)
        xP = np.stack([xsp[:, :, :, p, :, q].reshape(2, C, NK)
                       for p in range(2) for q in range(2)], axis=1)
        m = {
            "xP": _f32(xP.reshape(2, 16, 128, NK)),
            "wc": wc_pack,
            "srb": srb_pack,
            "onesc": np.ones((128, 1), np.float32),
            "wq": wq_pack,
            "wk": wk_pack,
            "wv": wv_pack,
            "wp": wp_pack,
            "uk": uk_pack,
            "uv": uv_pack,
        }
        in_maps.append(m)
    return in_maps


_NC_CACHE = None


def _get_nc():
    global _NC_CACHE
    if _NC_CACHE is None:
        _NC_CACHE = build_nc()
    return _NC_CACHE


def kernel(x_re, x_im, Wq, Wkv, Wproj, bproj, sr_w, sr_b, gain, bias, H, W):
    from concourse.bass_utils import run_bass_kernel_spmd

    nc = _get_nc()
    in_maps = host_prep(x_re, x_im, Wq, Wkv, Wproj, bproj, sr_w, sr_b,
                        gain, bias)
    res = run_bass_kernel_spmd(nc, in_maps, list(range(8)))
    bproj = np.asarray(bproj)
    out = np.zeros((B, N, C), dtype=np.complex64)
    for b in range(B):
        p0 = res.results[2 * b]["outT"]
        p1 = res.results[2 * b + 1]["outT"]
        acc = p0 + p1  # [2, 4cc, 5qc, 128, 512]
        accf = acc.transpose(0, 1, 3, 2, 4).reshape(2, 512, 5 * 512)[:, :, :N]
        full = np.empty((N, C), np.complex64)
        full[_PERM, :] = (accf[0] + 1j * accf[1]).T
        out[b] = full + bproj[None, :]
    return out
